# revision 5
# baseline (speedup 1.0000x reference)
"""FullAttention (non-standard multiplicative causal mask) on 8 TRN2 cores.

Reference (per batch b, head h):
    S = Q @ K^T                      [L, L]
    S = S * tril(ones)               (multiplicative mask: zeros above diag)
    A = softmax(S / sqrt(E))         (masked slots contribute exp(0)=1)
    O = A @ V

Key observation: for s > l, P[l,s] = exp(0) = 1, so
    num[l]   = sum_{s<=l} exp(z_ls) v_s  +  sum_{s>l} v_s
    denom[l] = sum_{s<=l} exp(z_ls)      +  (L-1-l)
The suffix terms depend only on tile(l), so they are added on the HOST
after gathering; the device only computes the causal (s-tile <= l-tile)
part, with the diagonal 128x128 blocks' upper triangles filled with 1.0.

Sharding: B*H = 32 (b,h) pairs -> 4 per core (2 "packs" of 2 heads).

Perf-critical structure (see trn2 tensor-engine doc):
  - Every matmul is 128-contraction so the PE never switches tiling
    mode (mode switches drain the array) and HAM stays warm (2.4 GHz).
    QK runs as 128-contraction by zero-padding: qt0 = [Q0; 0],
    qt1 = [0; Q1] against the shared stationary [K0; K1] -- the zero
    rows kill the wrong-head contribution.
  - Scores for both heads of a pack land in one [128, 2, 512] f32 PSUM
    pair-tile (2 adjacent banks); ONE ScalarE exp covers both heads
    (fixed ACT overhead amortized over 2x the elements).
  - Causal fixup (fill 1.0 above diagonal) on GpSimd, off the hot path.
  - Inputs stream in 512-column groups so the first QK piece only waits
    on ~0.5 MB; warmup matmuls cover that latency and pre-warm HAM.
  - PV emission lags QK by a global (cross-chunk) 2-piece queue so the
    PE and ScalarE never stall at chunk boundaries.
  - PV accumulates out^T [66, 512] per head over s-tiles; at chunk end
    a DVE copy moves it to SBUF and it is DMA'd out untransposed;
    the host does transpose + suffix add + denominator divide.
"""

import numpy as np

import concourse.bass as bass
import concourse.mybir as mybir
import concourse.tile as tile
from concourse import bacc

F32 = mybir.dt.float32
F16 = mybir.dt.float16
AF = mybir.ActivationFunctionType

B, L, H, E = 2, 2048, 16, 64
D = 64
SCALE = 0.125          # 1/sqrt(64)
NCORES = 8
BH_PER_CORE = (B * H) // NCORES   # 4
PACKS = BH_PER_CORE // 2          # 2
NT = L // 128                     # 16 s-tiles
NJ = L // 512                     # 4 l-chunks
VW = 66                           # [V | 1 | 0pad] (even moving width)
DELAY = 3                         # PV lags QK by this many pieces

_cached = None


def _build_program():
    nc = bacc.Bacc("TRN2", target_bir_lowering=False)
    qt = nc.dram_tensor("qt", [PACKS, 2, NJ, 128, 512], F16, kind="ExternalInput")
    kt = nc.dram_tensor("kt", [PACKS, NJ, 128, 512], F16, kind="ExternalInput")
    v1d = nc.dram_tensor("v1", [BH_PER_CORE, NJ, 128, 4 * VW], F16,
                         kind="ExternalInput")
    o = nc.dram_tensor("o", [BH_PER_CORE, NJ, VW, 512], F32, kind="ExternalOutput")

    with tile.TileContext(nc) as tc:
        with (
            tc.tile_pool(name="consts", bufs=1) as consts,
            tc.tile_pool(name="qk_sb", bufs=2) as qk_sb,
            tc.tile_pool(name="v1_sb", bufs=2) as v1_pool,
            tc.tile_pool(name="pt", bufs=8) as pt_pool,
            tc.tile_pool(name="osb", bufs=3) as o_pool,
            tc.tile_pool(name="pp_ps", bufs=3, space="PSUM") as pp_ps,
            tc.tile_pool(name="ot_ps", bufs=2, space="PSUM") as ot_ps,
        ):
            # ~2.6us of matmuls: cover the group-0 input DMA and warm HAM
            warm_sb = consts.tile([128, 512], F16)
            nc.gpsimd.memset(warm_sb, 0.25)
            warm_ps = pp_ps.tile([128, 2, 512], F32, tag="pp", name="warm")
            for w in range(10):
                nc.tensor.matmul(
                    warm_ps[:, 0, :], warm_sb[:, 0:128], warm_sb,
                    start=True, stop=True, skip_group_check=True,
                )

            pack_tiles = {}

            def load_pack(p):
                ktl, qtl, v1l = [], ([], []), ([], [])
                for g in range(NJ):
                    kt_t = qk_sb.tile([128, 512], F16, tag=f"kt{g}")
                    nc.sync.dma_start(out=kt_t, in_=kt[p, g])
                    ktl.append(kt_t)
                    for hh in range(2):
                        q_t = qk_sb.tile([128, 512], F16, tag=f"qt{hh}{g}")
                        eng = nc.sync if hh == 0 else nc.gpsimd
                        eng.dma_start(out=q_t, in_=qt[p, hh, g])
                        qtl[hh].append(q_t)
                    for hh in range(2):
                        v_t = v1_pool.tile([128, 4, VW], F16, tag=f"v1{hh}{g}")
                        eng = nc.sync if hh == 0 else nc.gpsimd
                        eng.dma_start(
                            out=v_t.rearrange("p t w -> p (t w)"),
                            in_=v1d[2 * p + hh, g],
                        )
                        v1l[hh].append(v_t)
                pack_tiles[p] = (ktl, qtl, v1l)

            pending = []

            def drain(to_len):
                while len(pending) > to_len:
                    emit, post = pending.pop(0)
                    emit()
                    if post is not None:
                        post()

            load_pack(0)
            for pack in range(PACKS):
                ktl, qtl, v1l = pack_tiles.pop(pack)

                for j in range(NJ):
                    if j == 2 and pack + 1 < PACKS:
                        load_pack(pack + 1)
                    lo = 512 * j
                    nk = 4 * j + 4          # s_tiles participating causally
                    ot_pair = [
                        ot_ps.tile([VW, 512], F32, tag="otr", name="ot")
                        for _ in range(2)
                    ]
                    started = [False, False]

                    korder = list(range(4 * j, nk)) + list(range(4 * j))
                    for ko, k in enumerate(korder):  # s_tile pieces, diag first

                        pp = pp_ps.tile([128, 2, 512], F32, tag="pp", name="pp")
                        pt_t = pt_pool.tile([128, 2, 512], F16, tag="pt", name="pt")
                        m = k - 4 * j               # >= 0 on diagonal pieces
                        qoff = 128 * max(0, m)
                        # QK both heads, shared stationary [K0; K1]
                        for hh in range(2):
                            nc.tensor.matmul(
                                pp[:, hh, qoff:512],
                                ktl[k // 4][:, 128 * (k % 4):128 * (k % 4 + 1)],
                                qtl[hh][j][:, qoff:512],
                                start=True, stop=True, skip_group_check=True,
                            )
                        # one exp for both heads (PSUM pair -> SBUF fp16)
                        nc.scalar.activation(
                            pt_t[:, :, qoff:512], pp[:, :, qoff:512],
                            AF.Exp, scale=SCALE,
                        )
                        if m >= 0:
                            # diagonal block: keep where l >= s else 1.0
                            for hh in range(2):
                                nc.gpsimd.affine_select(
                                    out=pt_t[:, hh, qoff:qoff + 128],
                                    in_=pt_t[:, hh, qoff:qoff + 128],
                                    compare_op=mybir.AluOpType.is_ge,
                                    fill=1.0,
                                    base=0,
                                    pattern=[[1, 128]],
                                    channel_multiplier=-1,
                                )

                        is_last = ko == nk - 1

                        def mk_pv(j=j, k=k, is_last=is_last, pt_t=pt_t,
                                  ot_pair=ot_pair, v1l=v1l, started=started):
                            def f():
                                poff = 128 * max(0, k - 4 * j)
                                for hh in range(2):
                                    nc.tensor.matmul(
                                        ot_pair[hh][:, poff:512],
                                        v1l[hh][k // 4][:, k % 4, :],
                                        pt_t[:, hh, poff:512],
                                        start=not started[hh],
                                        stop=is_last,
                                        skip_group_check=True,
                                    )
                                    started[hh] = True
                            return f

                        post = None
                        if is_last:
                            is_final = pack == PACKS - 1 and j == NJ - 1

                            def post(pack=pack, j=j, ot_pair=ot_pair,
                                     is_final=is_final):
                                for hh in range(2):
                                    bh = 2 * pack + hh
                                    ot_s = o_pool.tile([VW, 512], F32, tag="ots")
                                    if is_final and hh == 1:
                                        nc.scalar.copy(ot_s, ot_pair[hh])
                                    else:
                                        nc.vector.tensor_copy(ot_s, ot_pair[hh])
                                    eng = nc.sync if hh == 0 else nc.gpsimd
                                    eng.dma_start(out=o[bh, j], in_=ot_s)

                        pending.append((mk_pv(), post))
                        drain(DELAY)
            drain(0)

    nc.compile()
    return nc


def _get_program():
    global _cached
    if _cached is None:
        _cached = _build_program()
    return _cached


def _shard_inputs(queries, keys, values):
    # [B, L, H, E] -> [B, H, E, L] -> [BH, E, L]
    qT = np.ascontiguousarray(queries.transpose(0, 2, 3, 1)).reshape(B * H, E, L)
    kT = np.ascontiguousarray(keys.transpose(0, 2, 3, 1)).reshape(B * H, E, L)
    # [B, L, H, D] -> [BH, L, D]
    vv = np.ascontiguousarray(values.transpose(0, 2, 1, 3)).reshape(B * H, L, D)
    in_maps = []
    for c in range(NCORES):
        s = c * BH_PER_CORE
        kp = kT[s:s + BH_PER_CORE].reshape(PACKS, 128, NJ, 512)
        kp = kp.transpose(0, 2, 1, 3)                       # [P, NJ, 128, 512]
        qp = qT[s:s + BH_PER_CORE].reshape(PACKS, 2, 64, NJ, 512)
        qz = np.zeros((PACKS, 2, NJ, 128, 512), dtype=np.float16)
        qz[:, 0, :, 0:64] = qp[:, 0].transpose(0, 2, 1, 3)
        qz[:, 1, :, 64:128] = qp[:, 1].transpose(0, 2, 1, 3)
        vb = vv[s:s + BH_PER_CORE].reshape(BH_PER_CORE, NT, 128, D)
        v1h = np.zeros((BH_PER_CORE, 128, NT, VW), dtype=np.float16)
        v1h[:, :, :, 0:64] = vb.transpose(0, 2, 1, 3)
        v1h[:, :, :, 64] = 1.0
        # group s-tiles by 4: [BH, NJ, 128, 4*VW]
        v1g = v1h.reshape(BH_PER_CORE, 128, NJ, 4 * VW).transpose(0, 2, 1, 3)
        in_maps.append({
            "qt": qz,
            "kt": np.ascontiguousarray(kp).astype(np.float16),
            "v1": np.ascontiguousarray(v1g),
        })
    return in_maps


def _gather_outputs(results, values):
    # device output: [BH, NJ, VW, 512] f32; rows 0:64 = num^T, 64 = denom
    dev = np.concatenate([r["o"] for r in results], axis=0)  # [BH, 4, 66, 512]
    numT = dev[:, :, 0:D, :].transpose(0, 2, 1, 3).reshape(B * H, D, L)
    den_part = dev[:, :, D, :].reshape(B * H, L)

    vv = np.ascontiguousarray(values.transpose(0, 2, 1, 3)).reshape(B * H, L, D)
    ts = vv.reshape(B * H, NT, 128, D).sum(axis=2)          # [BH, NT, D]
    suf = np.flip(np.cumsum(np.flip(ts, 1), axis=1), 1)      # incl. own tile
    suf = np.concatenate([suf[:, 1:], np.zeros_like(suf[:, :1])], axis=1)

    tl = np.arange(L) // 128
    cnt = (L - 128 * (tl + 1)).astype(np.float32)
    num = numT.transpose(0, 2, 1) + suf[:, tl, :]            # [BH, L, D]
    den = den_part + cnt[None, :]
    out = num / den[:, :, None]
    return np.ascontiguousarray(
        out.reshape(B, H, L, D).transpose(0, 2, 1, 3)
    ).astype(np.float32)


def kernel(queries, keys, values, _trace=[False]):
    from concourse.bass_utils import run_bass_kernel_spmd

    queries = np.asarray(queries, dtype=np.float32)
    keys = np.asarray(keys, dtype=np.float32)
    values = np.asarray(values, dtype=np.float32)
    nc = _get_program()
    in_maps = _shard_inputs(queries, keys, values)
    res = run_bass_kernel_spmd(
        nc, in_maps, core_ids=list(range(NCORES)), trace=_trace[0]
    )
    out = _gather_outputs(res.results, values)
    if _trace[0]:
        kernel.last_results = res
    return out


# revision 6
# speedup vs baseline: 1.1756x; 1.1756x over previous
"""FullAttention (non-standard multiplicative causal mask) on 8 TRN2 cores.

Reference (per batch b, head h):
    S = Q @ K^T                      [L, L]
    S = S * tril(ones)               (multiplicative mask: zeros above diag)
    A = softmax(S / sqrt(E))         (masked slots contribute exp(0)=1)
    O = A @ V

Key observation: for s > l, P[l,s] = exp(0) = 1, so
    num[l]   = sum_{s<=l} exp(z_ls) v_s  +  sum_{s>l} v_s
    denom[l] = sum_{s<=l} exp(z_ls)      +  (L-1-l)
The suffix terms depend only on tile(l), so they are added on the HOST
after gathering; the device only computes the causal (s-tile <= l-tile)
part, with the diagonal 128x128 blocks' upper triangles filled with 1.0.

Sharding: B*H = 32 (b,h) pairs -> 4 per core (2 "packs" of 2 heads).

Perf-critical structure (see trn2 tensor-engine doc):
  - Every matmul is 128-contraction so the PE never switches tiling
    mode (mode switches drain the array) and HAM stays warm (2.4 GHz).
    QK runs as 128-contraction by zero-padding: qt0 = [Q0; 0],
    qt1 = [0; Q1] against the shared stationary [K0; K1] -- the zero
    rows kill the wrong-head contribution.
  - Scores for both heads of a pack land in one [128, 2, 512] f32 PSUM
    pair-tile (2 adjacent banks); ONE ScalarE exp covers both heads
    (fixed ACT overhead amortized over 2x the elements).
  - Causal fixup (fill 1.0 above diagonal) on GpSimd, off the hot path.
  - Inputs stream in 512-column groups so the first QK piece only waits
    on ~0.5 MB; warmup matmuls cover that latency and pre-warm HAM.
  - PV emission lags QK by a global (cross-chunk) 2-piece queue so the
    PE and ScalarE never stall at chunk boundaries.
  - PV accumulates out^T [66, 512] per head over s-tiles; at chunk end
    a DVE copy moves it to SBUF and it is DMA'd out untransposed;
    the host does transpose + suffix add + denominator divide.
"""

import numpy as np

import concourse.bass as bass
import concourse.mybir as mybir
import concourse.tile as tile
from concourse import bacc

F32 = mybir.dt.float32
F16 = mybir.dt.float16
AF = mybir.ActivationFunctionType

B, L, H, E = 2, 2048, 16, 64
D = 64
SCALE = 0.125          # 1/sqrt(64)
NCORES = 8
BH_PER_CORE = (B * H) // NCORES   # 4
PACKS = BH_PER_CORE // 2          # 2
NT = L // 128                     # 16 s-tiles
NJ = L // 512                     # 4 l-chunks
VW = 66                           # [V | 1 | 0pad] (even moving width)
DELAY = 2                         # PV lags QK by this many pieces

_cached = None


def _build_program():
    nc = bacc.Bacc("TRN2", target_bir_lowering=False)
    qt = nc.dram_tensor("qt", [PACKS, 2, NJ, 128, 512], F16, kind="ExternalInput")
    kt = nc.dram_tensor("kt", [PACKS, NJ, 128, 512], F16, kind="ExternalInput")
    v1d = nc.dram_tensor("v1", [BH_PER_CORE, NJ, 128, 4 * VW], F16,
                         kind="ExternalInput")
    o = nc.dram_tensor("o", [BH_PER_CORE, NJ, VW, 512], F32, kind="ExternalOutput")

    with tile.TileContext(nc) as tc:
        with (
            tc.tile_pool(name="consts", bufs=1) as consts,
            tc.tile_pool(name="qk_sb", bufs=2) as qk_sb,
            tc.tile_pool(name="v1_sb", bufs=2) as v1_pool,
            tc.tile_pool(name="pt", bufs=8) as pt_pool,
            tc.tile_pool(name="osb", bufs=3) as o_pool,
            tc.tile_pool(name="pp_ps", bufs=3, space="PSUM") as pp_ps,
            tc.tile_pool(name="ot_ps", bufs=2, space="PSUM") as ot_ps,
        ):
            # ~2.6us of matmuls: cover the group-0 input DMA and warm HAM
            warm_sb = consts.tile([128, 512], F16)
            nc.gpsimd.memset(warm_sb, 0.25)
            warm_ps = pp_ps.tile([128, 2, 512], F32, tag="pp", name="warm")
            for w in range(10):
                nc.tensor.matmul(
                    warm_ps[:, 0, :], warm_sb[:, 0:128], warm_sb,
                    start=True, stop=True, skip_group_check=True,
                )

            pack_tiles = {}

            def load_pack(p):
                ktl, qtl, v1l = [], ([], []), ([], [])
                for g in range(NJ):
                    kt_t = qk_sb.tile([128, 512], F16, tag=f"kt{g}")
                    nc.sync.dma_start(out=kt_t, in_=kt[p, g])
                    ktl.append(kt_t)
                    for hh in range(2):
                        q_t = qk_sb.tile([128, 512], F16, tag=f"qt{hh}{g}")
                        eng = nc.sync if hh == 0 else nc.gpsimd
                        eng.dma_start(out=q_t, in_=qt[p, hh, g])
                        qtl[hh].append(q_t)
                    for hh in range(2):
                        v_t = v1_pool.tile([128, 4, VW], F16, tag=f"v1{hh}{g}")
                        eng = nc.sync if hh == 0 else nc.gpsimd
                        eng.dma_start(
                            out=v_t.rearrange("p t w -> p (t w)"),
                            in_=v1d[2 * p + hh, g],
                        )
                        v1l[hh].append(v_t)
                pack_tiles[p] = (ktl, qtl, v1l)

            pending = []

            def drain(to_len):
                while len(pending) > to_len:
                    emit, post = pending.pop(0)
                    emit()
                    if post is not None:
                        post()

            load_pack(0)
            for pack in range(PACKS):
                ktl, qtl, v1l = pack_tiles.pop(pack)

                for j in range(NJ):
                    if j == 2 and pack + 1 < PACKS:
                        load_pack(pack + 1)
                    lo = 512 * j
                    nk = 4 * j + 4          # s_tiles participating causally
                    ot_pair = [
                        ot_ps.tile([VW, 512], F32, tag="otr", name="ot")
                        for _ in range(2)
                    ]
                    started = [False, False]

                    for k in range(nk):             # s_tile pieces

                        pp = pp_ps.tile([128, 2, 512], F32, tag="pp", name="pp")
                        pt_t = pt_pool.tile([128, 2, 512], F16, tag="pt", name="pt")
                        m = k - 4 * j               # >= 0 on diagonal pieces
                        qoff = 128 * max(0, m)
                        # QK both heads, shared stationary [K0; K1]
                        for hh in range(2):
                            nc.tensor.matmul(
                                pp[:, hh, qoff:512],
                                ktl[k // 4][:, 128 * (k % 4):128 * (k % 4 + 1)],
                                qtl[hh][j][:, qoff:512],
                                start=True, stop=True, skip_group_check=True,
                            )
                        # one exp for both heads (PSUM pair -> SBUF fp16)
                        nc.scalar.activation(
                            pt_t[:, :, qoff:512], pp[:, :, qoff:512],
                            AF.Exp, scale=SCALE,
                        )
                        if m >= 0:
                            # diagonal block: keep where l >= s else 1.0
                            for hh in range(2):
                                nc.gpsimd.affine_select(
                                    out=pt_t[:, hh, qoff:qoff + 128],
                                    in_=pt_t[:, hh, qoff:qoff + 128],
                                    compare_op=mybir.AluOpType.is_ge,
                                    fill=1.0,
                                    base=0,
                                    pattern=[[1, 128]],
                                    channel_multiplier=-1,
                                )

                        is_last = k == nk - 1

                        def mk_pv(j=j, k=k, is_last=is_last, pt_t=pt_t,
                                  ot_pair=ot_pair, v1l=v1l, started=started):
                            def f():
                                poff = 128 * max(0, k - 4 * j)
                                for hh in range(2):
                                    nc.tensor.matmul(
                                        ot_pair[hh][:, poff:512],
                                        v1l[hh][k // 4][:, k % 4, :],
                                        pt_t[:, hh, poff:512],
                                        start=not started[hh],
                                        stop=is_last,
                                        skip_group_check=True,
                                    )
                                    started[hh] = True
                            return f

                        post = None
                        if is_last:
                            is_final = pack == PACKS - 1 and j == NJ - 1

                            def post(pack=pack, j=j, ot_pair=ot_pair,
                                     is_final=is_final):
                                for hh in range(2):
                                    bh = 2 * pack + hh
                                    ot_s = o_pool.tile([VW, 512], F32, tag="ots")
                                    if is_final and hh == 1:
                                        nc.scalar.copy(ot_s, ot_pair[hh])
                                    else:
                                        nc.vector.tensor_copy(ot_s, ot_pair[hh])
                                    nc.sync.dma_start(out=o[bh, j], in_=ot_s)

                        pending.append((mk_pv(), post))
                        drain(DELAY)
            drain(0)

    nc.compile()
    return nc


def _get_program():
    global _cached
    if _cached is None:
        _cached = _build_program()
    return _cached


def _shard_inputs(queries, keys, values):
    # [B, L, H, E] -> [B, H, E, L] -> [BH, E, L]
    qT = np.ascontiguousarray(queries.transpose(0, 2, 3, 1)).reshape(B * H, E, L)
    kT = np.ascontiguousarray(keys.transpose(0, 2, 3, 1)).reshape(B * H, E, L)
    # [B, L, H, D] -> [BH, L, D]
    vv = np.ascontiguousarray(values.transpose(0, 2, 1, 3)).reshape(B * H, L, D)
    in_maps = []
    for c in range(NCORES):
        s = c * BH_PER_CORE
        kp = kT[s:s + BH_PER_CORE].reshape(PACKS, 128, NJ, 512)
        kp = kp.transpose(0, 2, 1, 3)                       # [P, NJ, 128, 512]
        qp = qT[s:s + BH_PER_CORE].reshape(PACKS, 2, 64, NJ, 512)
        qz = np.zeros((PACKS, 2, NJ, 128, 512), dtype=np.float16)
        qz[:, 0, :, 0:64] = qp[:, 0].transpose(0, 2, 1, 3)
        qz[:, 1, :, 64:128] = qp[:, 1].transpose(0, 2, 1, 3)
        vb = vv[s:s + BH_PER_CORE].reshape(BH_PER_CORE, NT, 128, D)
        v1h = np.zeros((BH_PER_CORE, 128, NT, VW), dtype=np.float16)
        v1h[:, :, :, 0:64] = vb.transpose(0, 2, 1, 3)
        v1h[:, :, :, 64] = 1.0
        # group s-tiles by 4: [BH, NJ, 128, 4*VW]
        v1g = v1h.reshape(BH_PER_CORE, 128, NJ, 4 * VW).transpose(0, 2, 1, 3)
        in_maps.append({
            "qt": qz,
            "kt": np.ascontiguousarray(kp).astype(np.float16),
            "v1": np.ascontiguousarray(v1g),
        })
    return in_maps


def _gather_outputs(results, values):
    # device output: [BH, NJ, VW, 512] f32; rows 0:64 = num^T, 64 = denom
    dev = np.concatenate([r["o"] for r in results], axis=0)  # [BH, 4, 66, 512]
    numT = dev[:, :, 0:D, :].transpose(0, 2, 1, 3).reshape(B * H, D, L)
    den_part = dev[:, :, D, :].reshape(B * H, L)

    vv = np.ascontiguousarray(values.transpose(0, 2, 1, 3)).reshape(B * H, L, D)
    ts = vv.reshape(B * H, NT, 128, D).sum(axis=2)          # [BH, NT, D]
    suf = np.flip(np.cumsum(np.flip(ts, 1), axis=1), 1)      # incl. own tile
    suf = np.concatenate([suf[:, 1:], np.zeros_like(suf[:, :1])], axis=1)

    tl = np.arange(L) // 128
    cnt = (L - 128 * (tl + 1)).astype(np.float32)
    num = numT.transpose(0, 2, 1) + suf[:, tl, :]            # [BH, L, D]
    den = den_part + cnt[None, :]
    out = num / den[:, :, None]
    return np.ascontiguousarray(
        out.reshape(B, H, L, D).transpose(0, 2, 1, 3)
    ).astype(np.float32)


def kernel(queries, keys, values, _trace=[False]):
    from concourse.bass_utils import run_bass_kernel_spmd

    queries = np.asarray(queries, dtype=np.float32)
    keys = np.asarray(keys, dtype=np.float32)
    values = np.asarray(values, dtype=np.float32)
    nc = _get_program()
    in_maps = _shard_inputs(queries, keys, values)
    res = run_bass_kernel_spmd(
        nc, in_maps, core_ids=list(range(NCORES)), trace=_trace[0]
    )
    out = _gather_outputs(res.results, values)
    if _trace[0]:
        kernel.last_results = res
    return out


# revision 8
# speedup vs baseline: 1.2277x; 1.0443x over previous
"""FullAttention (non-standard multiplicative causal mask) on 8 TRN2 cores.

Reference (per batch b, head h):
    S = Q @ K^T                      [L, L]
    S = S * tril(ones)               (multiplicative mask: zeros above diag)
    A = softmax(S / sqrt(E))         (masked slots contribute exp(0)=1)
    O = A @ V

Key observation: for s > l, P[l,s] = exp(0) = 1, so
    num[l]   = sum_{s<=l} exp(z_ls) v_s  +  sum_{s>l} v_s
    denom[l] = sum_{s<=l} exp(z_ls)      +  (L-1-l)
The suffix terms depend only on tile(l), so they are added on the HOST
after gathering; the device only computes the causal (s-tile <= l-tile)
part, with the diagonal 128x128 blocks' upper triangles filled with 1.0.

Sharding: B*H = 32 (b,h) pairs -> 4 per core (2 "packs" of 2 heads).

Perf-critical structure (see trn2 tensor-engine doc):
  - Every matmul is 128-contraction so the PE never switches tiling
    mode (mode switches drain the array) and HAM stays warm (2.4 GHz).
    QK runs as 128-contraction by zero-padding: qt0 = [Q0; 0],
    qt1 = [0; Q1] against the shared stationary [K0; K1] -- the zero
    rows kill the wrong-head contribution.
  - Scores for both heads of a pack land in one [128, 2, 512] f32 PSUM
    pair-tile (2 adjacent banks); ONE ScalarE exp covers both heads
    (fixed ACT overhead amortized over 2x the elements).
  - Causal fixup (fill 1.0 above diagonal) on GpSimd, off the hot path.
  - Inputs stream in 512-column groups so the first QK piece only waits
    on ~0.5 MB; warmup matmuls cover that latency and pre-warm HAM.
  - PV emission lags QK by a global (cross-chunk) 2-piece queue so the
    PE and ScalarE never stall at chunk boundaries.
  - PV accumulates out^T [66, 512] per head over s-tiles; at chunk end
    a DVE copy moves it to SBUF and it is DMA'd out untransposed;
    the host does transpose + suffix add + denominator divide.
"""

import numpy as np

import concourse.bass as bass
import concourse.mybir as mybir
import concourse.tile as tile
from concourse import bacc

F32 = mybir.dt.float32
F16 = mybir.dt.float16
AF = mybir.ActivationFunctionType

B, L, H, E = 2, 2048, 16, 64
D = 64
SCALE = 0.125          # 1/sqrt(64)
NCORES = 8
BH_PER_CORE = (B * H) // NCORES   # 4
PACKS = BH_PER_CORE // 2          # 2
NT = L // 128                     # 16 s-tiles
NJ = L // 512                     # 4 l-chunks
VW = 66                           # [V | 1 | 0pad] (even moving width)
DELAY = 3                         # PV lags QK by this many pieces

_cached = None


def _build_program():
    nc = bacc.Bacc("TRN2", target_bir_lowering=False)
    qt = nc.dram_tensor("qt", [PACKS, 2, NJ, 128, 512], F16, kind="ExternalInput")
    kt = nc.dram_tensor("kt", [PACKS, NJ, 128, 512], F16, kind="ExternalInput")
    v1d = nc.dram_tensor("v1", [BH_PER_CORE, NJ, 128, 4 * VW], F16,
                         kind="ExternalInput")
    o = nc.dram_tensor("o", [PACKS, NJ, VW, 2 * 512], F32, kind="ExternalOutput")

    with tile.TileContext(nc) as tc:
        with (
            tc.tile_pool(name="consts", bufs=1) as consts,
            tc.tile_pool(name="qk_sb", bufs=2) as qk_sb,
            tc.tile_pool(name="v1_sb", bufs=2) as v1_pool,
            tc.tile_pool(name="pt", bufs=8) as pt_pool,
            tc.tile_pool(name="osb", bufs=3) as o_pool,
            tc.tile_pool(name="pp_ps", bufs=3, space="PSUM") as pp_ps,
            tc.tile_pool(name="ot_ps", bufs=2, space="PSUM") as ot_ps,
        ):
            # ~2.6us of matmuls: cover the group-0 input DMA and warm HAM
            warm_sb = consts.tile([128, 512], F16)
            nc.gpsimd.memset(warm_sb, 0.25)
            warm_ps = pp_ps.tile([128, 2, 512], F32, tag="pp", name="warm")
            for w in range(10):
                nc.tensor.matmul(
                    warm_ps[:, 0, :], warm_sb[:, 0:128], warm_sb,
                    start=True, stop=True, skip_group_check=True,
                )

            pack_tiles = {}

            def load_pack(p):
                ktl = [None] * NJ
                qtl = ([None] * NJ, [None] * NJ)
                v1l = ([None] * NJ, [None] * NJ)
                for gi in range(NJ):
                    gq = NJ - 1 - gi            # qt groups j=3 first
                    kt_t = qk_sb.tile([128, 512], F16, tag=f"kt{gi}")
                    nc.sync.dma_start(out=kt_t, in_=kt[p, gi])
                    ktl[gi] = kt_t
                    for hh in range(2):
                        q_t = qk_sb.tile([128, 512], F16, tag=f"qt{hh}{gq}")
                        eng = nc.sync if hh == 0 else nc.gpsimd
                        eng.dma_start(out=q_t, in_=qt[p, hh, gq])
                        qtl[hh][gq] = q_t
                    for hh in range(2):
                        v_t = v1_pool.tile([128, 4, VW], F16, tag=f"v1{hh}{gi}")
                        eng = nc.sync if hh == 0 else nc.gpsimd
                        eng.dma_start(
                            out=v_t.rearrange("p t w -> p (t w)"),
                            in_=v1d[2 * p + hh, gi],
                        )
                        v1l[hh][gi] = v_t
                pack_tiles[p] = (ktl, qtl, v1l)

            pending = []

            def drain(to_len):
                while len(pending) > to_len:
                    emit, post = pending.pop(0)
                    emit()
                    if post is not None:
                        post()

            load_pack(0)
            for pack in range(PACKS):
                ktl, qtl, v1l = pack_tiles.pop(pack)

                for jo, j in enumerate([3, 2, 1, 0]):
                    if jo == 2 and pack + 1 < PACKS:
                        load_pack(pack + 1)
                    lo = 512 * j
                    nk = 4 * j + 4          # s_tiles participating causally
                    ot_pair = [
                        ot_ps.tile([VW, 512], F32, tag="otr", name="ot")
                        for _ in range(2)
                    ]
                    started = [False, False]

                    for k in range(nk):             # s_tile pieces

                        pp = pp_ps.tile([128, 2, 512], F32, tag="pp", name="pp")
                        pt_t = pt_pool.tile([128, 2, 512], F16, tag="pt", name="pt")
                        m = k - 4 * j               # >= 0 on diagonal pieces
                        qoff = 128 * max(0, m)
                        # QK both heads, shared stationary [K0; K1]
                        for hh in range(2):
                            nc.tensor.matmul(
                                pp[:, hh, qoff:512],
                                ktl[k // 4][:, 128 * (k % 4):128 * (k % 4 + 1)],
                                qtl[hh][j][:, qoff:512],
                                start=True, stop=True, skip_group_check=True,
                            )
                        # one exp for both heads (PSUM pair -> SBUF fp16)
                        nc.scalar.activation(
                            pt_t[:, :, qoff:512], pp[:, :, qoff:512],
                            AF.Exp, scale=SCALE,
                        )
                        if m >= 0:
                            # diagonal block: keep where l >= s else 1.0
                            for hh in range(2):
                                nc.gpsimd.affine_select(
                                    out=pt_t[:, hh, qoff:qoff + 128],
                                    in_=pt_t[:, hh, qoff:qoff + 128],
                                    compare_op=mybir.AluOpType.is_ge,
                                    fill=1.0,
                                    base=0,
                                    pattern=[[1, 128]],
                                    channel_multiplier=-1,
                                )

                        is_last = k == nk - 1

                        def mk_pv(j=j, k=k, is_last=is_last, pt_t=pt_t,
                                  ot_pair=ot_pair, v1l=v1l, started=started):
                            def f():
                                poff = 128 * max(0, k - 4 * j)
                                for hh in range(2):
                                    nc.tensor.matmul(
                                        ot_pair[hh][:, poff:512],
                                        v1l[hh][k // 4][:, k % 4, :],
                                        pt_t[:, hh, poff:512],
                                        start=not started[hh],
                                        stop=is_last,
                                        skip_group_check=True,
                                    )
                                    started[hh] = True
                            return f

                        post = None
                        if is_last:
                            is_final = pack == PACKS - 1 and jo == NJ - 1

                            def post(pack=pack, j=j, ot_pair=ot_pair,
                                     is_final=is_final):
                                ot_s = o_pool.tile([VW, 2, 512], F32, tag="ots")
                                for hh in range(2):
                                    if is_final and hh == 1:
                                        nc.scalar.copy(ot_s[:, hh, :], ot_pair[hh])
                                    else:
                                        nc.vector.tensor_copy(ot_s[:, hh, :],
                                                              ot_pair[hh])
                                    started_ = None
                                nc.sync.dma_start(
                                    out=o[pack, j],
                                    in_=ot_s.rearrange("p a b -> p (a b)"))

                        pending.append((mk_pv(), post))
                        drain(DELAY)
            drain(0)

    nc.compile()
    return nc


def _get_program():
    global _cached
    if _cached is None:
        _cached = _build_program()
    return _cached


def _shard_inputs(queries, keys, values):
    # [B, L, H, E] -> [B, H, E, L] -> [BH, E, L]
    qT = np.ascontiguousarray(queries.transpose(0, 2, 3, 1)).reshape(B * H, E, L)
    kT = np.ascontiguousarray(keys.transpose(0, 2, 3, 1)).reshape(B * H, E, L)
    # [B, L, H, D] -> [BH, L, D]
    vv = np.ascontiguousarray(values.transpose(0, 2, 1, 3)).reshape(B * H, L, D)
    in_maps = []
    for c in range(NCORES):
        s = c * BH_PER_CORE
        kp = kT[s:s + BH_PER_CORE].reshape(PACKS, 128, NJ, 512)
        kp = kp.transpose(0, 2, 1, 3)                       # [P, NJ, 128, 512]
        qp = qT[s:s + BH_PER_CORE].reshape(PACKS, 2, 64, NJ, 512)
        qz = np.zeros((PACKS, 2, NJ, 128, 512), dtype=np.float16)
        qz[:, 0, :, 0:64] = qp[:, 0].transpose(0, 2, 1, 3)
        qz[:, 1, :, 64:128] = qp[:, 1].transpose(0, 2, 1, 3)
        vb = vv[s:s + BH_PER_CORE].reshape(BH_PER_CORE, NT, 128, D)
        v1h = np.zeros((BH_PER_CORE, 128, NT, VW), dtype=np.float16)
        v1h[:, :, :, 0:64] = vb.transpose(0, 2, 1, 3)
        v1h[:, :, :, 64] = 1.0
        # group s-tiles by 4: [BH, NJ, 128, 4*VW]
        v1g = v1h.reshape(BH_PER_CORE, 128, NJ, 4 * VW).transpose(0, 2, 1, 3)
        in_maps.append({
            "qt": qz,
            "kt": np.ascontiguousarray(kp).astype(np.float16),
            "v1": np.ascontiguousarray(v1g),
        })
    return in_maps


def _gather_outputs(results, values):
    # device output: [PACKS, NJ, VW, 2, 512] f32 per core;
    # rows 0:64 = num^T, 64 = denom; dim 3 = head-in-pack
    dev = np.concatenate([r["o"] for r in results], axis=0)
    dev = dev.reshape(NCORES * PACKS, NJ, VW, 2, 512)
    dev = dev.transpose(0, 3, 1, 2, 4).reshape(B * H, NJ, VW, 512)
    numT = dev[:, :, 0:D, :].transpose(0, 2, 1, 3).reshape(B * H, D, L)
    den_part = dev[:, :, D, :].reshape(B * H, L)

    vv = np.ascontiguousarray(values.transpose(0, 2, 1, 3)).reshape(B * H, L, D)
    ts = vv.reshape(B * H, NT, 128, D).sum(axis=2)          # [BH, NT, D]
    suf = np.flip(np.cumsum(np.flip(ts, 1), axis=1), 1)      # incl. own tile
    suf = np.concatenate([suf[:, 1:], np.zeros_like(suf[:, :1])], axis=1)

    tl = np.arange(L) // 128
    cnt = (L - 128 * (tl + 1)).astype(np.float32)
    num = numT.transpose(0, 2, 1) + suf[:, tl, :]            # [BH, L, D]
    den = den_part + cnt[None, :]
    out = num / den[:, :, None]
    return np.ascontiguousarray(
        out.reshape(B, H, L, D).transpose(0, 2, 1, 3)
    ).astype(np.float32)


def kernel(queries, keys, values, _trace=[False]):
    from concourse.bass_utils import run_bass_kernel_spmd

    queries = np.asarray(queries, dtype=np.float32)
    keys = np.asarray(keys, dtype=np.float32)
    values = np.asarray(values, dtype=np.float32)
    nc = _get_program()
    in_maps = _shard_inputs(queries, keys, values)
    res = run_bass_kernel_spmd(
        nc, in_maps, core_ids=list(range(NCORES)), trace=_trace[0]
    )
    out = _gather_outputs(res.results, values)
    if _trace[0]:
        kernel.last_results = res
    return out


# revision 9
# speedup vs baseline: 1.2858x; 1.0474x over previous
"""FullAttention (non-standard multiplicative causal mask) on 8 TRN2 cores.

Reference (per batch b, head h):
    S = Q @ K^T                      [L, L]
    S = S * tril(ones)               (multiplicative mask: zeros above diag)
    A = softmax(S / sqrt(E))         (masked slots contribute exp(0)=1)
    O = A @ V

Key observation: for s > l, P[l,s] = exp(0) = 1, so
    num[l]   = sum_{s<=l} exp(z_ls) v_s  +  sum_{s>l} v_s
    denom[l] = sum_{s<=l} exp(z_ls)      +  (L-1-l)
The suffix terms depend only on tile(l), so they are added on the HOST
after gathering; the device only computes the causal (s-tile <= l-tile)
part, with the diagonal 128x128 blocks' upper triangles filled with 1.0.

Sharding: B*H = 32 (b,h) pairs -> 4 per core (2 "packs" of 2 heads).

Perf-critical structure (see trn2 tensor-engine doc):
  - Every matmul is 128-contraction so the PE never switches tiling
    mode (mode switches drain the array) and HAM stays warm (2.4 GHz).
    QK runs as 128-contraction by zero-padding: qt0 = [Q0; 0],
    qt1 = [0; Q1] against the shared stationary [K0; K1] -- the zero
    rows kill the wrong-head contribution.
  - Scores for both heads of a pack land in one [128, 2, 512] f32 PSUM
    pair-tile (2 adjacent banks); ONE ScalarE exp covers both heads
    (fixed ACT overhead amortized over 2x the elements).
  - Causal fixup (fill 1.0 above diagonal) on GpSimd, off the hot path.
  - Inputs stream in 512-column groups so the first QK piece only waits
    on ~0.5 MB; warmup matmuls cover that latency and pre-warm HAM.
  - PV emission lags QK by a global (cross-chunk) 2-piece queue so the
    PE and ScalarE never stall at chunk boundaries.
  - PV accumulates out^T [66, 512] per head over s-tiles; at chunk end
    a DVE copy moves it to SBUF and it is DMA'd out untransposed;
    the host does transpose + suffix add + denominator divide.
"""

import numpy as np

import concourse.bass as bass
import concourse.mybir as mybir
import concourse.tile as tile
from concourse import bacc

F32 = mybir.dt.float32
F16 = mybir.dt.float16
AF = mybir.ActivationFunctionType

B, L, H, E = 2, 2048, 16, 64
D = 64
SCALE = 0.125          # 1/sqrt(64)
NCORES = 8
BH_PER_CORE = (B * H) // NCORES   # 4
PACKS = BH_PER_CORE // 2          # 2
NT = L // 128                     # 16 s-tiles
NJ = L // 512                     # 4 l-chunks
VW = 66                           # [V | 1 | 0pad] (even moving width)
DELAY = 3                         # PV lags QK by this many pieces

_cached = None


def _build_program():
    nc = bacc.Bacc("TRN2", target_bir_lowering=False)
    qt = nc.dram_tensor("qt", [PACKS, 2, NJ, 128, 512], F16, kind="ExternalInput")
    kt = nc.dram_tensor("kt", [PACKS, NJ, 128, 512], F16, kind="ExternalInput")
    v1d = nc.dram_tensor("v1", [BH_PER_CORE, NJ, 128, 4 * VW], F16,
                         kind="ExternalInput")
    o = nc.dram_tensor("o", [PACKS, NJ, VW, 2 * 512], F32, kind="ExternalOutput")

    with tile.TileContext(nc) as tc:
        with (
            tc.tile_pool(name="consts", bufs=1) as consts,
            tc.tile_pool(name="qk_sb", bufs=2) as qk_sb,
            tc.tile_pool(name="v1_sb", bufs=2) as v1_pool,
            tc.tile_pool(name="pt", bufs=8) as pt_pool,
            tc.tile_pool(name="osb", bufs=3) as o_pool,
            tc.tile_pool(name="pp_ps", bufs=3, space="PSUM") as pp_ps,
            tc.tile_pool(name="ot_ps", bufs=2, space="PSUM") as ot_ps,
        ):
            # ~2.6us of matmuls: cover the group-0 input DMA and warm HAM
            warm_sb = consts.tile([128, 512], F16)
            nc.gpsimd.memset(warm_sb, 0.25)
            warm_ps = pp_ps.tile([128, 2, 512], F32, tag="pp", name="warm")
            for w in range(10):
                nc.tensor.matmul(
                    warm_ps[:, 0, :], warm_sb[:, 0:128], warm_sb,
                    start=True, stop=True, skip_group_check=True,
                )

            pack_tiles = {}

            def load_pack(p):
                ktl = [None] * NJ
                qtl = ([None] * NJ, [None] * NJ)
                v1l = ([None] * NJ, [None] * NJ)
                for gi in range(NJ):
                    gq = NJ - 1 - gi            # qt groups j=3 first
                    kt_t = qk_sb.tile([128, 512], F16, tag=f"kt{gi}")
                    nc.sync.dma_start(out=kt_t, in_=kt[p, gi])
                    ktl[gi] = kt_t
                    for hh in range(2):
                        q_t = qk_sb.tile([128, 512], F16, tag=f"qt{hh}{gq}")
                        eng = nc.sync if hh == 0 else nc.gpsimd
                        eng.dma_start(out=q_t, in_=qt[p, hh, gq])
                        qtl[hh][gq] = q_t
                    for hh in range(2):
                        v_t = v1_pool.tile([128, 4, VW], F16, tag=f"v1{hh}{gi}")
                        eng = nc.sync if hh == 0 else nc.gpsimd
                        eng.dma_start(
                            out=v_t.rearrange("p t w -> p (t w)"),
                            in_=v1d[2 * p + hh, gi],
                        )
                        v1l[hh][gi] = v_t
                pack_tiles[p] = (ktl, qtl, v1l)

            pending = []

            def drain(to_len):
                while len(pending) > to_len:
                    emit, post = pending.pop(0)
                    emit()
                    if post is not None:
                        post()

            load_pack(0)
            for pack in range(PACKS):
                ktl, qtl, v1l = pack_tiles.pop(pack)

                for jo, j in enumerate([3, 2, 1, 0]):
                    if jo == 2 and pack + 1 < PACKS:
                        load_pack(pack + 1)
                    lo = 512 * j
                    nk = 4 * j + 4          # s_tiles participating causally
                    ot_pair = [
                        ot_ps.tile([VW, 512], F32, tag="otr", name="ot")
                        for _ in range(2)
                    ]
                    started = [False, False]

                    for k in range(nk):             # s_tile pieces

                        pp = pp_ps.tile([128, 2, 512], F32, tag="pp", name="pp")
                        pt_t = pt_pool.tile([128, 2, 512], F16, tag="pt", name="pt")
                        m = k - 4 * j               # >= 0 on diagonal pieces
                        qoff = 128 * max(0, m)
                        # QK both heads, shared stationary [K0; K1]
                        for hh in range(2):
                            nc.tensor.matmul(
                                pp[:, hh, qoff:512],
                                ktl[k // 4][:, 128 * (k % 4):128 * (k % 4 + 1)],
                                qtl[hh][j][:, qoff:512],
                                start=True, stop=True, skip_group_check=True,
                            )
                        # one exp for both heads (PSUM pair -> SBUF fp16)
                        nc.scalar.activation(
                            pt_t[:, :, qoff:512], pp[:, :, qoff:512],
                            AF.Exp, scale=SCALE,
                        )
                        if m >= 0:
                            # diagonal block: keep where l >= s else 1.0
                            for hh in range(2):
                                nc.gpsimd.affine_select(
                                    out=pt_t[:, hh, qoff:qoff + 128],
                                    in_=pt_t[:, hh, qoff:qoff + 128],
                                    compare_op=mybir.AluOpType.is_ge,
                                    fill=1.0,
                                    base=0,
                                    pattern=[[1, 128]],
                                    channel_multiplier=-1,
                                )

                        is_last = k == nk - 1

                        def mk_pv(j=j, k=k, is_last=is_last, pt_t=pt_t,
                                  ot_pair=ot_pair, v1l=v1l, started=started):
                            def f():
                                poff = 128 * max(0, k - 4 * j)
                                for hh in range(2):
                                    nc.tensor.matmul(
                                        ot_pair[hh][:, poff:512],
                                        v1l[hh][k // 4][:, k % 4, :],
                                        pt_t[:, hh, poff:512],
                                        start=not started[hh],
                                        stop=is_last,
                                        skip_group_check=True,
                                    )
                                    started[hh] = True
                            return f

                        post = None
                        if is_last:
                            is_final = pack == PACKS - 1 and jo == NJ - 1

                            def post(pack=pack, j=j, ot_pair=ot_pair,
                                     is_final=is_final):
                                ot_s = o_pool.tile([VW, 2, 512], F32, tag="ots")
                                for hh in range(2):
                                    if is_final and hh == 1:
                                        nc.scalar.copy(ot_s[:, hh, :], ot_pair[hh])
                                    else:
                                        nc.vector.tensor_copy(ot_s[:, hh, :],
                                                              ot_pair[hh])
                                nc.sync.dma_start(
                                    out=o[pack, j],
                                    in_=ot_s.rearrange("p a b -> p (a b)"))

                        pending.append((mk_pv(), post))
                        drain(DELAY)
            drain(0)

    nc.compile()
    return nc


def _get_program():
    global _cached
    if _cached is None:
        _cached = _build_program()
    return _cached


def _shard_inputs(queries, keys, values):
    # [B, L, H, E] -> [B, H, E, L] -> [BH, E, L]
    qT = np.ascontiguousarray(queries.transpose(0, 2, 3, 1)).reshape(B * H, E, L)
    kT = np.ascontiguousarray(keys.transpose(0, 2, 3, 1)).reshape(B * H, E, L)
    # [B, L, H, D] -> [BH, L, D]
    vv = np.ascontiguousarray(values.transpose(0, 2, 1, 3)).reshape(B * H, L, D)
    in_maps = []
    for c in range(NCORES):
        s = c * BH_PER_CORE
        kp = kT[s:s + BH_PER_CORE].reshape(PACKS, 128, NJ, 512)
        kp = kp.transpose(0, 2, 1, 3)                       # [P, NJ, 128, 512]
        qp = qT[s:s + BH_PER_CORE].reshape(PACKS, 2, 64, NJ, 512)
        qz = np.zeros((PACKS, 2, NJ, 128, 512), dtype=np.float16)
        qz[:, 0, :, 0:64] = qp[:, 0].transpose(0, 2, 1, 3)
        qz[:, 1, :, 64:128] = qp[:, 1].transpose(0, 2, 1, 3)
        vb = vv[s:s + BH_PER_CORE].reshape(BH_PER_CORE, NT, 128, D)
        v1h = np.zeros((BH_PER_CORE, 128, NT, VW), dtype=np.float16)
        v1h[:, :, :, 0:64] = vb.transpose(0, 2, 1, 3)
        v1h[:, :, :, 64] = 1.0
        # group s-tiles by 4: [BH, NJ, 128, 4*VW]
        v1g = v1h.reshape(BH_PER_CORE, 128, NJ, 4 * VW).transpose(0, 2, 1, 3)
        in_maps.append({
            "qt": qz,
            "kt": np.ascontiguousarray(kp).astype(np.float16),
            "v1": np.ascontiguousarray(v1g),
        })
    return in_maps


def _gather_outputs(results, values):
    # device output: [PACKS, NJ, VW, 2, 512] f32 per core;
    # rows 0:64 = num^T, 64 = denom; dim 3 = head-in-pack
    dev = np.concatenate([r["o"] for r in results], axis=0)
    dev = dev.reshape(NCORES * PACKS, NJ, VW, 2, 512)
    dev = dev.transpose(0, 3, 1, 2, 4).reshape(B * H, NJ, VW, 512)
    numT = dev[:, :, 0:D, :].transpose(0, 2, 1, 3).reshape(B * H, D, L)
    den_part = dev[:, :, D, :].reshape(B * H, L)

    vv = np.ascontiguousarray(values.transpose(0, 2, 1, 3)).reshape(B * H, L, D)
    ts = vv.reshape(B * H, NT, 128, D).sum(axis=2)          # [BH, NT, D]
    suf = np.flip(np.cumsum(np.flip(ts, 1), axis=1), 1)      # incl. own tile
    suf = np.concatenate([suf[:, 1:], np.zeros_like(suf[:, :1])], axis=1)

    tl = np.arange(L) // 128
    cnt = (L - 128 * (tl + 1)).astype(np.float32)
    num = numT.transpose(0, 2, 1) + suf[:, tl, :]            # [BH, L, D]
    den = den_part + cnt[None, :]
    out = num / den[:, :, None]
    return np.ascontiguousarray(
        out.reshape(B, H, L, D).transpose(0, 2, 1, 3)
    ).astype(np.float32)


def kernel(queries, keys, values, _trace=[False]):
    from concourse.bass_utils import run_bass_kernel_spmd

    queries = np.asarray(queries, dtype=np.float32)
    keys = np.asarray(keys, dtype=np.float32)
    values = np.asarray(values, dtype=np.float32)
    nc = _get_program()
    in_maps = _shard_inputs(queries, keys, values)
    res = run_bass_kernel_spmd(
        nc, in_maps, core_ids=list(range(NCORES)), trace=_trace[0]
    )
    out = _gather_outputs(res.results, values)
    if _trace[0]:
        kernel.last_results = res
    return out


# revision 10
# speedup vs baseline: 1.2965x; 1.0083x over previous
"""FullAttention (non-standard multiplicative causal mask) on 8 TRN2 cores.

Reference (per batch b, head h):
    S = Q @ K^T                      [L, L]
    S = S * tril(ones)               (multiplicative mask: zeros above diag)
    A = softmax(S / sqrt(E))         (masked slots contribute exp(0)=1)
    O = A @ V

Key observation: for s > l, P[l,s] = exp(0) = 1, so
    num[l]   = sum_{s<=l} exp(z_ls) v_s  +  sum_{s>l} v_s
    denom[l] = sum_{s<=l} exp(z_ls)      +  (L-1-l)
The suffix terms depend only on tile(l), so they are added on the HOST
after gathering; the device only computes the causal (s-tile <= l-tile)
part, with the diagonal 128x128 blocks' upper triangles filled with 1.0.

Sharding: B*H = 32 (b,h) pairs -> 4 per core (2 "packs" of 2 heads).

Perf-critical structure (see trn2 tensor-engine doc):
  - Every matmul is 128-contraction so the PE never switches tiling
    mode (mode switches drain the array) and HAM stays warm (2.4 GHz).
    QK runs as 128-contraction by zero-padding: qt0 = [Q0; 0],
    qt1 = [0; Q1] against the shared stationary [K0; K1] -- the zero
    rows kill the wrong-head contribution.
  - Scores for both heads of a pack land in one [128, 2, 512] f32 PSUM
    pair-tile (2 adjacent banks); ONE ScalarE exp covers both heads
    (fixed ACT overhead amortized over 2x the elements).
  - Causal fixup (fill 1.0 above diagonal) on GpSimd, off the hot path.
  - Inputs stream in 512-column groups so the first QK piece only waits
    on ~0.5 MB; warmup matmuls cover that latency and pre-warm HAM.
  - PV emission lags QK by a global (cross-chunk) 2-piece queue so the
    PE and ScalarE never stall at chunk boundaries.
  - PV accumulates out^T [66, 512] per head over s-tiles; at chunk end
    a DVE copy moves it to SBUF and it is DMA'd out untransposed;
    the host does transpose + suffix add + denominator divide.
"""

import numpy as np

import concourse.bass as bass
import concourse.mybir as mybir
import concourse.tile as tile
from concourse import bacc

F32 = mybir.dt.float32
F16 = mybir.dt.float16
AF = mybir.ActivationFunctionType

B, L, H, E = 2, 2048, 16, 64
D = 64
SCALE = 0.125          # 1/sqrt(64)
NCORES = 8
BH_PER_CORE = (B * H) // NCORES   # 4
PACKS = BH_PER_CORE // 2          # 2
NT = L // 128                     # 16 s-tiles
NJ = L // 512                     # 4 l-chunks
VW = 66                           # [V | 1 | 0pad] (even moving width)
DELAY = 4                         # PV lags QK by this many pieces

_cached = None


def _build_program():
    nc = bacc.Bacc("TRN2", target_bir_lowering=False)
    qt = nc.dram_tensor("qt", [PACKS, 2, NJ, 128, 512], F16, kind="ExternalInput")
    kt = nc.dram_tensor("kt", [PACKS, NJ, 128, 512], F16, kind="ExternalInput")
    v1d = nc.dram_tensor("v1", [BH_PER_CORE, NJ, 128, 4 * VW], F16,
                         kind="ExternalInput")
    o = nc.dram_tensor("o", [PACKS, NJ, VW, 2 * 512], F32, kind="ExternalOutput")

    with tile.TileContext(nc) as tc:
        with (
            tc.tile_pool(name="consts", bufs=1) as consts,
            tc.tile_pool(name="qk_sb", bufs=2) as qk_sb,
            tc.tile_pool(name="v1_sb", bufs=2) as v1_pool,
            tc.tile_pool(name="pt", bufs=8) as pt_pool,
            tc.tile_pool(name="osb", bufs=3) as o_pool,
            tc.tile_pool(name="pp_ps", bufs=3, space="PSUM") as pp_ps,
            tc.tile_pool(name="ot_ps", bufs=2, space="PSUM") as ot_ps,
        ):
            # ~2.6us of matmuls: cover the group-0 input DMA and warm HAM
            warm_sb = consts.tile([128, 512], F16)
            nc.gpsimd.memset(warm_sb, 0.25)
            warm_ps = pp_ps.tile([128, 2, 512], F32, tag="pp", name="warm")
            tbl_sb = consts.tile([128, 2], F16)
            nc.scalar.activation(tbl_sb, warm_sb[:, 0:2], AF.Exp, scale=0.125)
            for w in range(10):
                nc.tensor.matmul(
                    warm_ps[:, 0, :], warm_sb[:, 0:128], warm_sb,
                    start=True, stop=True, skip_group_check=True,
                )

            pack_tiles = {}

            def load_pack(p):
                ktl = [None] * NJ
                qtl = ([None] * NJ, [None] * NJ)
                v1l = ([None] * NJ, [None] * NJ)
                for gi in range(NJ):
                    gq = NJ - 1 - gi            # qt groups j=3 first
                    kt_t = qk_sb.tile([128, 512], F16, tag=f"kt{gi}")
                    nc.sync.dma_start(out=kt_t, in_=kt[p, gi])
                    ktl[gi] = kt_t
                    for hh in range(2):
                        q_t = qk_sb.tile([128, 512], F16, tag=f"qt{hh}{gq}")
                        eng = nc.sync if hh == 0 else nc.gpsimd
                        eng.dma_start(out=q_t, in_=qt[p, hh, gq])
                        qtl[hh][gq] = q_t
                    for hh in range(2):
                        v_t = v1_pool.tile([128, 4, VW], F16, tag=f"v1{hh}{gi}")
                        eng = nc.sync if hh == 0 else nc.gpsimd
                        eng.dma_start(
                            out=v_t.rearrange("p t w -> p (t w)"),
                            in_=v1d[2 * p + hh, gi],
                        )
                        v1l[hh][gi] = v_t
                pack_tiles[p] = (ktl, qtl, v1l)

            pending = []

            def drain(to_len):
                while len(pending) > to_len:
                    emit, post = pending.pop(0)
                    emit()
                    if post is not None:
                        post()

            load_pack(0)
            for pack in range(PACKS):
                ktl, qtl, v1l = pack_tiles.pop(pack)

                for jo, j in enumerate([3, 2, 1, 0]):
                    if jo == 2 and pack + 1 < PACKS:
                        load_pack(pack + 1)
                    lo = 512 * j
                    nk = 4 * j + 4          # s_tiles participating causally
                    ot_pair = [
                        ot_ps.tile([VW, 512], F32, tag="otr", name="ot")
                        for _ in range(2)
                    ]
                    started = [False, False]

                    for k in range(nk):             # s_tile pieces

                        pp = pp_ps.tile([128, 2, 512], F32, tag="pp", name="pp")
                        pt_t = pt_pool.tile([128, 2, 512], F16, tag="pt", name="pt")
                        m = k - 4 * j               # >= 0 on diagonal pieces
                        qoff = 128 * max(0, m)
                        # QK both heads, shared stationary [K0; K1]
                        for hh in range(2):
                            nc.tensor.matmul(
                                pp[:, hh, qoff:512],
                                ktl[k // 4][:, 128 * (k % 4):128 * (k % 4 + 1)],
                                qtl[hh][j][:, qoff:512],
                                start=True, stop=True, skip_group_check=True,
                            )
                        # one exp for both heads (PSUM pair -> SBUF fp16)
                        nc.scalar.activation(
                            pt_t[:, :, qoff:512], pp[:, :, qoff:512],
                            AF.Exp, scale=SCALE,
                        )
                        if m >= 0:
                            # diagonal block: keep where l >= s else 1.0
                            for hh in range(2):
                                nc.gpsimd.affine_select(
                                    out=pt_t[:, hh, qoff:qoff + 128],
                                    in_=pt_t[:, hh, qoff:qoff + 128],
                                    compare_op=mybir.AluOpType.is_ge,
                                    fill=1.0,
                                    base=0,
                                    pattern=[[1, 128]],
                                    channel_multiplier=-1,
                                )

                        is_last = k == nk - 1

                        def mk_pv(j=j, k=k, is_last=is_last, pt_t=pt_t,
                                  ot_pair=ot_pair, v1l=v1l, started=started):
                            def f():
                                poff = 128 * max(0, k - 4 * j)
                                for hh in range(2):
                                    nc.tensor.matmul(
                                        ot_pair[hh][:, poff:512],
                                        v1l[hh][k // 4][:, k % 4, :],
                                        pt_t[:, hh, poff:512],
                                        start=not started[hh],
                                        stop=is_last,
                                        skip_group_check=True,
                                    )
                                    started[hh] = True
                            return f

                        post = None
                        if is_last:
                            is_final = pack == PACKS - 1 and jo == NJ - 1

                            def post(pack=pack, j=j, ot_pair=ot_pair,
                                     is_final=is_final):
                                ot_s = o_pool.tile([VW, 2, 512], F32, tag="ots")
                                for hh in range(2):
                                    if is_final and hh == 1:
                                        nc.scalar.copy(ot_s[:, hh, :], ot_pair[hh])
                                    else:
                                        nc.vector.tensor_copy(ot_s[:, hh, :],
                                                              ot_pair[hh])
                                nc.sync.dma_start(
                                    out=o[pack, j],
                                    in_=ot_s.rearrange("p a b -> p (a b)"))

                        pending.append((mk_pv(), post))
                        drain(DELAY)
            drain(0)

    nc.compile()
    return nc


def _get_program():
    global _cached
    if _cached is None:
        _cached = _build_program()
    return _cached


def _shard_inputs(queries, keys, values):
    # [B, L, H, E] -> [B, H, E, L] -> [BH, E, L]
    qT = np.ascontiguousarray(queries.transpose(0, 2, 3, 1)).reshape(B * H, E, L)
    kT = np.ascontiguousarray(keys.transpose(0, 2, 3, 1)).reshape(B * H, E, L)
    # [B, L, H, D] -> [BH, L, D]
    vv = np.ascontiguousarray(values.transpose(0, 2, 1, 3)).reshape(B * H, L, D)
    in_maps = []
    for c in range(NCORES):
        s = c * BH_PER_CORE
        kp = kT[s:s + BH_PER_CORE].reshape(PACKS, 128, NJ, 512)
        kp = kp.transpose(0, 2, 1, 3)                       # [P, NJ, 128, 512]
        qp = qT[s:s + BH_PER_CORE].reshape(PACKS, 2, 64, NJ, 512)
        qz = np.zeros((PACKS, 2, NJ, 128, 512), dtype=np.float16)
        qz[:, 0, :, 0:64] = qp[:, 0].transpose(0, 2, 1, 3)
        qz[:, 1, :, 64:128] = qp[:, 1].transpose(0, 2, 1, 3)
        vb = vv[s:s + BH_PER_CORE].reshape(BH_PER_CORE, NT, 128, D)
        v1h = np.zeros((BH_PER_CORE, 128, NT, VW), dtype=np.float16)
        v1h[:, :, :, 0:64] = vb.transpose(0, 2, 1, 3)
        v1h[:, :, :, 64] = 1.0
        # group s-tiles by 4: [BH, NJ, 128, 4*VW]
        v1g = v1h.reshape(BH_PER_CORE, 128, NJ, 4 * VW).transpose(0, 2, 1, 3)
        in_maps.append({
            "qt": qz,
            "kt": np.ascontiguousarray(kp).astype(np.float16),
            "v1": np.ascontiguousarray(v1g),
        })
    return in_maps


def _gather_outputs(results, values):
    # device output: [PACKS, NJ, VW, 2, 512] f32 per core;
    # rows 0:64 = num^T, 64 = denom; dim 3 = head-in-pack
    dev = np.concatenate([r["o"] for r in results], axis=0)
    dev = dev.reshape(NCORES * PACKS, NJ, VW, 2, 512)
    dev = dev.transpose(0, 3, 1, 2, 4).reshape(B * H, NJ, VW, 512)
    numT = dev[:, :, 0:D, :].transpose(0, 2, 1, 3).reshape(B * H, D, L)
    den_part = dev[:, :, D, :].reshape(B * H, L)

    vv = np.ascontiguousarray(values.transpose(0, 2, 1, 3)).reshape(B * H, L, D)
    ts = vv.reshape(B * H, NT, 128, D).sum(axis=2)          # [BH, NT, D]
    suf = np.flip(np.cumsum(np.flip(ts, 1), axis=1), 1)      # incl. own tile
    suf = np.concatenate([suf[:, 1:], np.zeros_like(suf[:, :1])], axis=1)

    tl = np.arange(L) // 128
    cnt = (L - 128 * (tl + 1)).astype(np.float32)
    num = numT.transpose(0, 2, 1) + suf[:, tl, :]            # [BH, L, D]
    den = den_part + cnt[None, :]
    out = num / den[:, :, None]
    return np.ascontiguousarray(
        out.reshape(B, H, L, D).transpose(0, 2, 1, 3)
    ).astype(np.float32)


def kernel(queries, keys, values, _trace=[False]):
    from concourse.bass_utils import run_bass_kernel_spmd

    queries = np.asarray(queries, dtype=np.float32)
    keys = np.asarray(keys, dtype=np.float32)
    values = np.asarray(values, dtype=np.float32)
    nc = _get_program()
    in_maps = _shard_inputs(queries, keys, values)
    res = run_bass_kernel_spmd(
        nc, in_maps, core_ids=list(range(NCORES)), trace=_trace[0]
    )
    out = _gather_outputs(res.results, values)
    if _trace[0]:
        kernel.last_results = res
    return out
